# revision 2
# baseline (speedup 1.0000x reference)
"""ContextualConv2d Trainium2 kernel (paired-tap, bf16).

Problem: grouped 3x3 conv (N=32, 128ci -> 256co, groups=4, 56x56, pad 1)
plus per-(batch,channel) context bias: out = conv(x, w) + (c @ cwT)[n,co]
+ bias[co].

Sharding (8 cores): core = (group-pair gp in {0,1}) x (batch quarter q in
{0..3}). Each core computes 8 images x 64 in-ch x 128 out-channels.

PE matmul cost on this target is N_free cycles regardless of K/M, so the
lever is matmuls-per-output-tile. The 9 taps are covered with 5 matmuls
(vs 9) by pairing two taps per matmul in the contraction dim: SBUF holds
the image twice -- partitions 0..63 plain, 64..127 pre-shifted -- so one
K=128 matmul with block-diagonal weights accumulates two taps at once:
  T1 = [x | x>>1col]:              AP(kh,0) -> taps (kh,0)+(kh,1)  (3 mm)
  T2 = [x>>2col | x>>(1row,2col)]: AP(0,0)  -> taps (0,2)+(1,2)    (1 mm)
                                   AP(1,0)  -> tap  (2,2)          (1 mm)
T1 comes from HBM (host-prepped, 2x replicated); T2 is built on-chip by
DVE copies from T1 (same-partition shifts), so x HBM traffic stays low.
All conv I/O is bf16 (fp32 PSUM accumulate; rel err ~1e-3), halving DMA
bytes. Epilogue fuses the context/bias add with the PSUM->SBUF downcast,
alternating ACT/DVE; y is DMA'd as bf16 and upcast on the host.
"""

import numpy as np
import ml_dtypes

from concourse import bass, mybir, tile
from concourse.vector_clock import ScopedClock
from concourse.bass_utils import run_bass_kernel_spmd

N, CIN, H, W = 32, 128, 56, 56
COUT, KH, KW = 256, 3, 3
GROUPS = 4
CDIM = 64
HP, WPL = H + 2, W + 1  # SBUF tile line: 58 rows x 57 cols
ROWS = 8              # output rows per n-tile
NT = H // ROWS        # 7 n-tiles per image
NFREE = ROWS * W      # 448 <= 512 fp32 PSUM bank limit
N_CORES = 8
IMGS = N // 4         # 8 images per core
CI = CIN // 2         # 64 input channels per core (2 groups)
CO = COUT // 2        # 128 output channels per core (2 groups)

# taps (kh, kw) per matmul for (half-a = partitions 0..63, half-b = 64..127)
TAPS = [
    ((0, 0), (0, 1)),  # M0: T1, AP row +0
    ((1, 0), (1, 1)),  # M1: T1, AP row +1
    ((2, 0), (2, 1)),  # M2: T1, AP row +2
    ((0, 2), (1, 2)),  # M3: T2, AP row +0
    (None, (2, 2)),    # M4: T2, AP row +1 (half-a weights zeroed)
]


class _TC(tile.TileContext):
    """This container's walrus accepts only one sem wait on a Drain
    (CTRL) instruction; TileContext's tail drain aggregates one wait per
    outstanding semaphore. Split them across sequential drains."""

    def _drain_and_barrier(self, tick_clock, wait_clock):
        drain_inst = self.nc.sync.drain()
        wait_clock.add_sem_waits(
            drain_inst.ins, ScopedClock({None: tick_clock.global_clock})
        )
        si = drain_inst.ins.sync_info
        if si is not None and len(si.on_wait) > 1:
            waits = list(si.on_wait)
            si.on_wait.clear()
            si.on_wait.append(waits[0])
            for w in waits[1:]:
                d2 = self.nc.sync.drain()
                d2.ins.sync_info = mybir.SyncInfo(on_wait=[w], on_update=[])
        self.nc.all_engine_barrier()
        assert self.sems is not None
        popped = self.nc._tile_sem_poison_stack.pop()
        assert popped is self._sem_poison
        self.nc.clear_and_free_semaphores(list(self.sems.allocated().values()))
        self.nc.all_engine_barrier()


_ws_ctr = [0]


def _split_waits(nc):
    """Walrus here caps sem waits at one per instruction; hoist extras
    onto injected same-engine NoOps placed just before the owner."""
    for fn in nc.m.functions:
        for blk in fn.blocks:
            insts = blk.instructions
            out = []
            changed = False
            for inst in insts:
                si = getattr(inst, "sync_info", None)
                if si is not None and si.on_wait and len(si.on_wait) > 1:
                    waits = list(si.on_wait)
                    for w in waits[:-1]:
                        _ws_ctr[0] += 1
                        out.append(
                            mybir.InstNoOp(
                                name=f"WSNOP-{_ws_ctr[0]}",
                                engine=inst.engine,
                                ins=[],
                                outs=[],
                                sync_info=mybir.SyncInfo(on_wait=[w], on_update=[]),
                                debug=inst.debug,
                            )
                        )
                        changed = True
                    si.on_wait.clear()
                    si.on_wait.append(waits[-1])
                out.append(inst)
            if changed:
                insts.clear()
                insts.extend(out)
    return nc


def build_program(loop_n: int = 0):
    """loop_n > 0 builds a benchmark variant: the conv body repeats
    loop_n times inside a hardware For_i so device time dominates the
    (RPC/transfer-heavy) wall clock. loop_n=0 is the production kernel."""
    f32 = mybir.dt.float32
    f32r = mybir.dt.float32r
    bf16 = mybir.dt.bfloat16
    nc = bass.Bass("TRN2", target_bir_lowering=False, debug=False)
    xs = nc.declare_dram_parameter("xs", [IMGS, 128, HP, WPL], bf16, isOutput=False)
    wb = nc.declare_dram_parameter("wb", [128, 5, CO], bf16, isOutput=False)
    cwb = nc.declare_dram_parameter("cwb", [CDIM + 1, CO], f32r, isOutput=False)
    cb = nc.declare_dram_parameter("cb", [CDIM + 1, IMGS], f32r, isOutput=False)
    y = nc.declare_dram_parameter("y", [IMGS, CO, H, W], bf16, isOutput=True)

    with _TC(nc) as tc:
        with (
            tc.tile_pool(name="wp", bufs=1) as wpool,
            tc.tile_pool(name="xp", bufs=3) as xpool,
            tc.tile_pool(name="x2p", bufs=3) as x2pool,
            tc.tile_pool(name="op", bufs=4) as opool,
            tc.tile_pool(name="psp", bufs=6, space="PSUM") as pspool,
            tc.tile_pool(name="psc", bufs=1, space="PSUM") as pscpool,
        ):
            wt = wpool.tile([128, 5, CO], bf16)
            nc.sync.dma_start(wt[:], wb[:])
            cwbt = wpool.tile([CDIM + 1, CO], f32r)
            nc.sync.dma_start(cwbt[:], cwb[:])
            cbt = wpool.tile([CDIM + 1, IMGS], f32r)
            nc.sync.dma_start(cbt[:], cb[:])

            # bctx[co, n] = sum_d c_weight[co,d] c[n,d] + bias[co]
            psc = pscpool.tile([CO, IMGS], f32)
            nc.tensor.matmul(psc[:, :], cwbt[:], cbt[:], start=True, stop=True)
            bctx = wpool.tile([CO, IMGS], f32)
            nc.vector.tensor_copy(bctx[:], psc[:, :])

            def conv_body():
                for img in range(IMGS):
                    xt = xpool.tile([128, HP, WPL], bf16, name=f"xt{img}", tag="xt")
                    nc.sync.dma_start(xt[:], xs[img])
                    x2 = x2pool.tile([128, HP, WPL], bf16, name=f"x2{img}", tag="x2")
                    # T2a = x>>2col (plus zeroed pad column), T2b = x>>(1row,2col)
                    nc.vector.memset(x2[0:64, :, 55:56], 0.0)
                    nc.vector.tensor_copy(x2[0:64, :, 0:55], xt[0:64, :, 2:57])
                    nc.vector.tensor_copy(
                        x2[64:128, 0:57, 0:56], xt[64:128, 1:58, 1:57]
                    )
                    ot = opool.tile([128, H * W], bf16, name=f"ot{img}", tag="ot")
                    for t in range(NT):
                        ps = pspool.tile(
                            [128, NFREE], f32, name=f"ps{img}_{t}", tag="ps"
                        )
                        h0 = t * ROWS
                        for m in range(5):
                            src = xt if m < 3 else x2
                            dh = m if m < 3 else m - 3
                            nc.tensor.matmul(
                                ps[:, :],
                                wt[:, m, :],
                                src[:, h0 + dh : h0 + dh + ROWS, 0:56],
                                start=(m == 0),
                                stop=(m == 4),
                            )
                        o = ot[:, t * NFREE : (t + 1) * NFREE]
                        if t % 2 == 0:
                            nc.scalar.activation(
                                o, ps[:, :], mybir.ActivationFunctionType.Identity,
                                bias=bctx[:, img : img + 1],
                            )
                        else:
                            nc.vector.tensor_scalar_add(
                                o, ps[:, :], bctx[:, img : img + 1]
                            )
                    nc.sync.dma_start(y[img].rearrange("c h w -> c (h w)"), ot[:])

            if loop_n > 0:
                with tc.For_i(0, loop_n, 1, hint_engines=(mybir.EngineType.PE,)):
                    conv_body()
            else:
                conv_body()
    _split_waits(nc)
    return nc


_prog_cache = {}


def _get_program():
    if "nc" not in _prog_cache:
        _prog_cache["nc"] = build_program()
    return _prog_cache["nc"]


def _shard_inputs(x, c, weight, bias, c_weight):
    """Build the per-core input dicts (pure layout prep, no math)."""
    bf16 = ml_dtypes.bfloat16
    xpad = np.zeros((N, CIN, H + 2, W + 2), np.float32)
    xpad[:, :, 1 : H + 1, 1 : W + 1] = x

    # Position-major block-diagonal paired-tap weights per group pair.
    wbs = []
    cwbs = []
    for gp in range(2):
        wsl = weight[CO * gp : CO * gp + CO]  # [128co, 32ci, 3, 3]
        blk = np.zeros((128, 5, CO), np.float32)
        for m, (ta, tb) in enumerate(TAPS):
            for s, tap in ((0, ta), (1, tb)):
                if tap is None:
                    continue
                kh, kw = tap
                for g in range(2):
                    blk[
                        s * 64 + g * 32 : s * 64 + g * 32 + 32,
                        m,
                        g * 64 : g * 64 + 64,
                    ] = wsl[g * 64 : g * 64 + 64, :, kh, kw].T
        wbs.append(blk.astype(bf16))

        cwbv = np.empty((CDIM + 1, CO), np.float32)
        cwbv[:CDIM] = c_weight[CO * gp : CO * gp + CO].T
        cwbv[CDIM] = bias[CO * gp : CO * gp + CO]
        cwbs.append(cwbv)

    in_maps = []
    for core in range(N_CORES):
        gp, q = divmod(core, 4)
        xc = xpad[
            IMGS * q : IMGS * q + IMGS, CI * gp : CI * gp + CI
        ]  # [8, 64, 58, 58]
        xall = np.empty((IMGS, 128, HP, WPL), np.float32)
        xall[:, 0:64] = xc[:, :, :, 0:57]   # T1a: x
        xall[:, 64:128] = xc[:, :, :, 1:58]  # T1b: x >> 1 col
        cbv = np.empty((CDIM + 1, IMGS), np.float32)
        cbv[:CDIM] = c[IMGS * q : IMGS * q + IMGS].T
        cbv[CDIM] = 1.0
        in_maps.append(
            {
                "xs": xall.astype(bf16),
                "wb": wbs[gp],
                "cwb": cwbs[gp],
                "cb": cbv,
            }
        )
    return in_maps


def kernel(x, c, weight, bias, c_weight):
    x = np.asarray(x, np.float32)
    c = np.asarray(c, np.float32)
    weight = np.asarray(weight, np.float32)
    bias = np.asarray(bias, np.float32)
    c_weight = np.asarray(c_weight, np.float32)

    nc = _get_program()
    in_maps = _shard_inputs(x, c, weight, bias, c_weight)
    res = run_bass_kernel_spmd(nc, in_maps, list(range(N_CORES)), trace=False)

    out = np.empty((N, COUT, H, W), np.float32)
    for core in range(N_CORES):
        gp, q = divmod(core, 4)
        out[IMGS * q : IMGS * q + IMGS, CO * gp : CO * gp + CO] = res.results[core][
            "y"
        ].astype(np.float32)
    return out


# revision 13
# speedup vs baseline: 1.3053x; 1.3053x over previous
"""ContextualConv2d Trainium2 kernel (paired-tap, bf16).

Problem: grouped 3x3 conv (N=32, 128ci -> 256co, groups=4, 56x56, pad 1)
plus per-(batch,channel) context bias: out = conv(x, w) + (c @ cwT)[n,co]
+ bias[co].

Sharding (8 cores): core = (group-pair gp in {0,1}) x (batch quarter q in
{0..3}). Each core computes 8 images x 64 in-ch x 128 out-channels.

PE matmul cost on this target is N_free cycles regardless of K/M, so the
lever is matmuls-per-output-tile. The 9 taps are covered with 5 matmuls
(vs 9) by pairing two taps per matmul in the contraction dim: SBUF holds
the image twice -- partitions 0..63 plain, 64..127 pre-shifted -- so one
K=128 matmul with block-diagonal weights accumulates two taps at once:
  T1 = [x | x>>1col]:              AP(kh,0) -> taps (kh,0)+(kh,1)  (3 mm)
  T2 = [x>>2col | x>>(1row,2col)]: AP(0,0)  -> taps (0,2)+(1,2)    (1 mm)
                                   AP(1,0)  -> tap  (2,2)          (1 mm)
T1 comes from HBM (host-prepped, 2x replicated); T2 is built on-chip by
DVE copies from T1 (same-partition shifts), so x HBM traffic stays low.
All conv I/O is bf16 (fp32 PSUM accumulate; rel err ~1e-3), halving DMA
bytes. Epilogue fuses the context/bias add with the PSUM->SBUF downcast,
alternating ACT/DVE; y is DMA'd as bf16 and upcast on the host.
"""

import numpy as np
import ml_dtypes

from concourse import bass, mybir, tile
from concourse.vector_clock import ScopedClock
from concourse.bass_utils import run_bass_kernel_spmd

N, CIN, H, W = 32, 128, 56, 56
COUT, KH, KW = 256, 3, 3
GROUPS = 4
CDIM = 64
HP, WPL = H + 2, W + 1  # SBUF tile line: 58 rows x 57 cols
ROWS = 8              # output rows per n-tile
NT = H // ROWS        # 7 n-tiles per image
NFREE = ROWS * W      # 448 <= 512 fp32 PSUM bank limit
N_CORES = 8
IMGS = N // 4         # 8 images per core
CI = CIN // 2         # 64 input channels per core (2 groups)
CO = COUT // 2        # 128 output channels per core (2 groups)

# taps (kh, kw) per matmul for (half-a = partitions 0..63, half-b = 64..127)
TAPS = [
    ((0, 0), (0, 1)),  # M0: T1, AP row +0
    ((1, 0), (1, 1)),  # M1: T1, AP row +1
    ((2, 0), (2, 1)),  # M2: T1, AP row +2
    ((0, 2), (1, 2)),  # M3: T2, AP row +0
    (None, (2, 2)),    # M4: T2, AP row +1 (half-a weights zeroed)
]


class _TC(tile.TileContext):
    """This container's walrus accepts only one sem wait on a Drain
    (CTRL) instruction; TileContext's tail drain aggregates one wait per
    outstanding semaphore. Split them across sequential drains."""

    def _drain_and_barrier(self, tick_clock, wait_clock):
        drain_inst = self.nc.sync.drain()
        wait_clock.add_sem_waits(
            drain_inst.ins, ScopedClock({None: tick_clock.global_clock})
        )
        si = drain_inst.ins.sync_info
        if si is not None and len(si.on_wait) > 1:
            waits = list(si.on_wait)
            si.on_wait.clear()
            si.on_wait.append(waits[0])
            for w in waits[1:]:
                d2 = self.nc.sync.drain()
                d2.ins.sync_info = mybir.SyncInfo(on_wait=[w], on_update=[])
        self.nc.all_engine_barrier()
        assert self.sems is not None
        popped = self.nc._tile_sem_poison_stack.pop()
        assert popped is self._sem_poison
        self.nc.clear_and_free_semaphores(list(self.sems.allocated().values()))
        self.nc.all_engine_barrier()


_ws_ctr = [0]


def _split_waits(nc):
    """Walrus here caps sem waits at one per instruction; hoist extras
    onto injected same-engine NoOps placed just before the owner."""
    for fn in nc.m.functions:
        for blk in fn.blocks:
            insts = blk.instructions
            out = []
            changed = False
            for inst in insts:
                si = getattr(inst, "sync_info", None)
                if si is not None and si.on_wait and len(si.on_wait) > 1:
                    waits = list(si.on_wait)
                    for w in waits[:-1]:
                        _ws_ctr[0] += 1
                        out.append(
                            mybir.InstNoOp(
                                name=f"WSNOP-{_ws_ctr[0]}",
                                engine=inst.engine,
                                ins=[],
                                outs=[],
                                sync_info=mybir.SyncInfo(on_wait=[w], on_update=[]),
                                debug=inst.debug,
                            )
                        )
                        changed = True
                    si.on_wait.clear()
                    si.on_wait.append(waits[-1])
                out.append(inst)
            if changed:
                insts.clear()
                insts.extend(out)
    return nc


def build_program(loop_n: int = 0):
    """loop_n > 0 builds a benchmark variant: the conv body repeats
    loop_n times inside a hardware For_i so device time dominates the
    (RPC/transfer-heavy) wall clock. loop_n=0 is the production kernel."""
    f32 = mybir.dt.float32
    f32r = mybir.dt.float32r
    bf16 = mybir.dt.bfloat16
    nc = bass.Bass("TRN2", target_bir_lowering=False, debug=False)
    xs = nc.declare_dram_parameter("xs", [IMGS, 128, HP, WPL], bf16, isOutput=False)
    wb = nc.declare_dram_parameter("wb", [128, 5, CO], bf16, isOutput=False)
    cwc = nc.declare_dram_parameter(
        "cwc", [CDIM + 1, CO + IMGS], f32r, isOutput=False
    )
    y = nc.declare_dram_parameter("y", [IMGS, CO, H, W], bf16, isOutput=True)

    with _TC(nc) as tc:
        with (
            tc.tile_pool(name="wp", bufs=1) as wpool,
            tc.tile_pool(name="xp", bufs=3) as xpool,
            tc.tile_pool(name="x2p", bufs=3) as x2pool,
            tc.tile_pool(name="op", bufs=4) as opool,
            tc.tile_pool(name="psp", bufs=6, space="PSUM") as pspool,
            tc.tile_pool(name="psc", bufs=1, space="PSUM") as pscpool,
        ):
            XB = 4  # x ring depth
            xring = [
                xpool.tile([128, HP, WPL], bf16, name=f"xr{r}") for r in range(XB)
            ]
            x2ring = [
                x2pool.tile([128, HP, WPL], bf16, name=f"x2r{r}") for r in range(XB)
            ]

            def prefetch(j):
                """DMA image j's T1 and derive T2 by on-chip shifted copies."""
                xt, x2 = xring[j % XB], x2ring[j % XB]
                nc.sync.dma_start(xt[:], xs[j])
                nc.vector.tensor_copy(x2[0:64, :, 0:55], xt[0:64, :, 2:57])
                nc.vector.tensor_copy(
                    x2[64:128, 0:57, 0:56], xt[64:128, 1:58, 1:57]
                )

            # Tiny context DMA first: its matmul then runs ~3.5us before the
            # first conv matmul, which completes the PE p-state ramp so conv
            # matmuls start at full clock. Image 0's big DMA goes next, ahead
            # of the weight DMA's fixed overheads.
            cwct = wpool.tile([CDIM + 1, CO + IMGS], f32r)
            nc.sync.dma_start(cwct[:], cwc[:])
            prefetch(0)
            wt = wpool.tile([128, 5, CO], bf16)
            nc.sync.dma_start(wt[:], wb[:])
            # T2a's zero pad column is write-once per ring buffer: the
            # per-image copies never touch it.
            for r in range(XB):
                nc.vector.memset(x2ring[r][0:64, :, 55:56], 0.0)

            # bctx[co, n] = sum_d c_weight[co,d] c[n,d] + bias[co]
            psc = pscpool.tile([CO, IMGS], f32)
            nc.tensor.matmul(
                psc[:, :], cwct[:, 0:CO], cwct[:, CO : CO + IMGS],
                start=True, stop=True,
            )
            bctx = wpool.tile([CO, IMGS], f32)
            nc.vector.tensor_copy(bctx[:], psc[:, :])

            def conv_body(looped: bool = True):
                for img in range(IMGS):
                    # Software-pipelined prefetch: issue the NEXT image's x
                    # DMA (wrapping into the next loop iteration) before this
                    # image's matmuls, so PE never waits at the boundary.
                    nxt = (img + 1) % IMGS
                    if looped or img < IMGS - 1:
                        prefetch(nxt)
                    xt, x2 = xring[img % XB], x2ring[img % XB]
                    ot = opool.tile([128, H * W], bf16, name=f"ot{img}", tag="ot")
                    for t in range(NT):
                        ps = pspool.tile(
                            [128, NFREE], f32, name=f"ps{img}_{t}", tag="ps"
                        )
                        h0 = t * ROWS
                        for m in range(5):
                            src = xt if m < 3 else x2
                            dh = m if m < 3 else m - 3
                            nc.tensor.matmul(
                                ps[:, :],
                                wt[:, m, :],
                                src[:, h0 + dh : h0 + dh + ROWS, 0:56],
                                start=(m == 0),
                                stop=(m == 4),
                            )
                        o = ot[:, t * NFREE : (t + 1) * NFREE]
                        if t % 2 == 0:
                            nc.scalar.activation(
                                o, ps[:, :], mybir.ActivationFunctionType.Identity,
                                bias=bctx[:, img : img + 1],
                            )
                        else:
                            nc.vector.tensor_scalar_add(
                                o, ps[:, :], bctx[:, img : img + 1]
                            )
                        if t == 3:
                            # First-half writeback overlaps the back-half
                            # compute; the split also shortens the tail drain.
                            yv = y[img].rearrange("c h w -> c (h w)")
                            nc.sync.dma_start(
                                yv[:, : 4 * NFREE], ot[:, : 4 * NFREE]
                            )
                    nc.sync.dma_start(yv[:, 4 * NFREE :], ot[:, 4 * NFREE :])

            if loop_n > 0:
                with tc.For_i(0, loop_n, 1, hint_engines=(mybir.EngineType.PE,)):
                    conv_body(looped=True)
            else:
                conv_body(looped=False)
    _split_waits(nc)
    return nc


_prog_cache = {}


def _get_program():
    if "nc" not in _prog_cache:
        _prog_cache["nc"] = build_program()
    return _prog_cache["nc"]


def _shard_inputs(x, c, weight, bias, c_weight):
    """Build the per-core input dicts (pure layout prep, no math)."""
    bf16 = ml_dtypes.bfloat16
    xpad = np.zeros((N, CIN, H + 2, W + 2), np.float32)
    xpad[:, :, 1 : H + 1, 1 : W + 1] = x

    # Position-major block-diagonal paired-tap weights per group pair.
    wbs = []
    cwbs = []
    for gp in range(2):
        wsl = weight[CO * gp : CO * gp + CO]  # [128co, 32ci, 3, 3]
        blk = np.zeros((128, 5, CO), np.float32)
        for m, (ta, tb) in enumerate(TAPS):
            for s, tap in ((0, ta), (1, tb)):
                if tap is None:
                    continue
                kh, kw = tap
                for g in range(2):
                    blk[
                        s * 64 + g * 32 : s * 64 + g * 32 + 32,
                        m,
                        g * 64 : g * 64 + 64,
                    ] = wsl[g * 64 : g * 64 + 64, :, kh, kw].T
        wbs.append(blk.astype(bf16))

        cwbv = np.empty((CDIM + 1, CO), np.float32)
        cwbv[:CDIM] = c_weight[CO * gp : CO * gp + CO].T
        cwbv[CDIM] = bias[CO * gp : CO * gp + CO]
        cwbs.append(cwbv)  # context weights; batch columns appended per core

    in_maps = []
    for core in range(N_CORES):
        gp, q = divmod(core, 4)
        xc = xpad[
            IMGS * q : IMGS * q + IMGS, CI * gp : CI * gp + CI
        ]  # [8, 64, 58, 58]
        xall = np.empty((IMGS, 128, HP, WPL), np.float32)
        xall[:, 0:64] = xc[:, :, :, 0:57]   # T1a: x
        xall[:, 64:128] = xc[:, :, :, 1:58]  # T1b: x >> 1 col
        cwcv = np.empty((CDIM + 1, CO + IMGS), np.float32)
        cwcv[:, :CO] = cwbs[gp]
        cwcv[:CDIM, CO:] = c[IMGS * q : IMGS * q + IMGS].T
        cwcv[CDIM, CO:] = 1.0
        in_maps.append(
            {
                "xs": xall.astype(bf16),
                "wb": wbs[gp],
                "cwc": cwcv,
            }
        )
    return in_maps


def kernel(x, c, weight, bias, c_weight):
    x = np.asarray(x, np.float32)
    c = np.asarray(c, np.float32)
    weight = np.asarray(weight, np.float32)
    bias = np.asarray(bias, np.float32)
    c_weight = np.asarray(c_weight, np.float32)

    nc = _get_program()
    in_maps = _shard_inputs(x, c, weight, bias, c_weight)
    res = run_bass_kernel_spmd(nc, in_maps, list(range(N_CORES)), trace=False)

    out = np.empty((N, COUT, H, W), np.float32)
    for core in range(N_CORES):
        gp, q = divmod(core, 4)
        out[IMGS * q : IMGS * q + IMGS, CO * gp : CO * gp + CO] = res.results[core][
            "y"
        ].astype(np.float32)
    return out
